# revision 1
# baseline (speedup 1.0000x reference)
"""Trainium2 Bass kernel for the 3-layer difflogic net (nn_Net_48610439856713).

~238us HW exec (baseline: 532us).  Full batch 512 on the free dim, 8-way
neuron sharding (2000 gates/core/layer), fp8 e4m3 activations.

Structure:
- Layers 1+2 FUSED: x is replicated, so each core evaluates the L1 values
  its own L2 gates consume, in consumption order, via one-hot PE matmuls
  (fp8 stationaries x fp8 x -> PSUM f32).  This eliminates both the L1
  AllGather and the L2 dma_gather; the only collective boundary is L2->L3.
- L2->L3: 2-chunk 8-rank AllGather (Shared output) overlapped with the
  producer's combine; L3 gates are host-bucketed by max source chunk
  (data-driven capacities) so gathers start after AG chunk 0 lands.
  L3 gathers run as 4 dma_gather calls on 4 distinct SWDGE queues
  (queue reuse while a prior call drains is pathological; <=4 calls).
- Combine h' = c0 + c1*a + c2*b + c3*ab per [128, 512] j-column, spread
  over engines: Act does v = c2*b + c0 (per-partition scale/bias APs) and
  the PSUM b-copy; DVE does the two STTs; the final t+w add goes to
  GpSimd during L1/L2 (idle there) and DVE during L3.
- Gather-output buffers are pre-touched with a copy reading h2 so the
  tile scheduler cannot hoist gathers ahead of the L12 combines on the
  in-order GpSimd queue (head-of-line blocking on the AllGather).
- Group-sum via accumulating PE matmuls (ones^T @ h3[:, j]), final tiny
  AllReduce; host only transposes the [3, 512] result.

Host-side work is integer/layout bookkeeping only (slot layouts, index
relabeling, one-hot packing, int16 wrapping); all float arithmetic
(softmax coefficients, combines, sums) runs on device.  Accuracy with
fp8 inter-layer activations: 2.9e-4 max rel err (gate: 2e-2).
"""

import os
import numpy as np

P = 128
B = 512
NCORE = 8
IN = 193
XROWS = 256
TAU = 100.0

GPC = 2000
NJ12 = 16
NS12 = NJ12 * P
L3N = 15999
SPG = 5333
NJ3 = 18
JPG3 = 6
NS3 = NJ3 * P

NCH = 2                     # AG chunks for the L2->L3 boundary (8 j-cols)
JCH = NJ12 // NCH

_CACHE = {}


def _l3_call_plan(caps3):
    """caps3: per-group j-cols per level (sum JPG3). Returns list of
    (level, j_list) gather calls; level-1 split in two for queue overlap."""
    base = np.concatenate([[0], np.cumsum(caps3)[:-1]])
    calls = []
    for lvl in range(NCH):
        js = []
        for g in range(3):
            js.extend(range(g * JPG3 + base[lvl], g * JPG3 + base[lvl] + caps3[lvl]))
        if not js:
            continue
        if len(js) > 6:
            third = (len(js) + 2) // 3
            for i in range(0, len(js), third):
                calls.append((lvl, js[i:i + third]))
        else:
            calls.append((lvl, js))
    return calls


def _build_nc(caps3):
    L3_CALLS = _l3_call_plan(caps3)
    import concourse.bacc as bacc
    import concourse.tile as tile
    import concourse.mybir as mybir

    f32 = mybir.dt.float32
    bf16 = mybir.dt.bfloat16
    f8 = mybir.dt.float8e4
    i16 = mybir.dt.int16
    Alu = mybir.AluOpType
    Act = mybir.ActivationFunctionType
    Ax = mybir.AxisListType

    nc = bacc.Bacc("TRN2", target_bir_lowering=False, debug=False,
                   num_devices=NCORE, num_swdge_queues=4)

    # ---- I/O ----
    xT_d = nc.dram_tensor("xT", [XROWS, B], f32, kind="ExternalInput")
    # one-hot stationaries: 32 eval-cols x 2 streams x 2 kchunks x 128 cols
    oh_d = nc.dram_tensor("oh", [P, 128 * P], f8, kind="ExternalInput")
    w1f = nc.dram_tensor("w1f", [P, 32 * 16], f32, kind="ExternalInput")
    w2p = nc.dram_tensor("w2p", [P, NJ12 * 16], f32, kind="ExternalInput")
    w3p = nc.dram_tensor("w3p", [P, NJ3 * 16], f32, kind="ExternalInput")
    i3_d = nc.dram_tensor("i3", [P, 2 * NS3 // 16], i16, kind="ExternalInput")
    out_d = nc.dram_tensor("out", [1, 3 * B], f32, kind="ExternalOutput")

    h2ch = [nc.dram_tensor(f"h2c{k}", [P, JCH * B], f8, kind="Internal")
            for k in range(NCH)]
    g2 = nc.dram_tensor("g2", [NCH * NCORE * P, JCH * B], f8,
                        kind="Internal", addr_space="Shared")
    win = nc.dram_tensor("win", [1, 16], f32, kind="Internal")
    warm = nc.dram_tensor("warm", [NCORE, 16], f32, kind="Internal")
    pin = nc.dram_tensor("pin", [1, 3 * B], f32, kind="Internal")
    pout = nc.dram_tensor("pout", [1, 3 * B], f32, kind="Internal")

    g8 = [list(range(NCORE))]

    with tile.TileContext(nc) as tc:
        with (
            tc.tile_pool(name="big", bufs=1) as big,
            tc.tile_pool(name="coef", bufs=1) as coef,
            tc.tile_pool(name="tmp", bufs=1) as tmp,
            tc.tile_pool(name="ab", bufs=1) as abp,
            tc.tile_pool(name="psum", bufs=1, space="PSUM") as psum,
        ):
            # ---- warm-up collective ----
            wsb = coef.tile([1, 16], f32, tag="wsb")
            nc.vector.memset(wsb[:], 0.0)
            nc.sync.dma_start(win[:], wsb[:])
            nc.gpsimd.collective_compute(
                "AllGather", Alu.bypass, replica_groups=g8,
                ins=[win[:]], outs=[warm[:]],
            )

            # ---- loads ----
            xt0 = big.tile([P, B], f32, tag="xt0")
            nc.sync.dma_start(xt0[:], xT_d[0:P, :])
            xt1 = big.tile([P, B], f32, tag="xt1")
            nc.sync.dma_start(xt1[:], xT_d[P:XROWS, :])
            x8i = big.tile([P, 2, B], f8, tag="x8i")
            nc.vector.tensor_copy(x8i[:, 0], xt0[:])
            nc.vector.tensor_copy(x8i[:, 1], xt1[:])
            ohs = big.tile([P, 128 * P], f8, tag="ohs")
            for q in range(4):
                nc.sync.dma_start(ohs[:, q * 32 * P:(q + 1) * 32 * P],
                                  oh_d[:, q * 32 * P:(q + 1) * 32 * P])

            def coeffs(wp, nj, tag):
                wt = tmp.tile([P, nj * 16], f32, tag=f"wt{tag}")
                nc.sync.dma_start(wt[:], wp[:])
                e = tmp.tile([P, nj * 16], f32, tag=f"e{tag}")
                nc.scalar.activation(e[:], wt[:], Act.Exp)
                e3 = e[:].rearrange("p (j g) -> p j g", g=16)
                e4 = e[:].rearrange("p (j h q) -> p j h q", h=4, q=4)
                ssum = coef.tile([P, nj], f32, tag=f"ss{tag}")
                nc.vector.reduce_sum(ssum[:], e3, axis=Ax.X)
                r = coef.tile([P, nj], f32, tag=f"r{tag}")
                nc.vector.reciprocal(r[:], ssum[:])
                c0 = coef.tile([P, nj], f32, tag=f"c0{tag}")
                c1 = coef.tile([P, nj], f32, tag=f"c1{tag}")
                c2 = coef.tile([P, nj], f32, tag=f"c2{tag}")
                c3 = coef.tile([P, nj], f32, tag=f"c3{tag}")
                nc.vector.reduce_sum(c0[:], e4[:, :, 2:4, :], axis=Ax.XY)
                t1 = tmp.tile([P, nj], f32, tag=f"t1{tag}")
                t2 = tmp.tile([P, nj], f32, tag=f"t2{tag}")
                nc.vector.reduce_sum(t1[:], e4[:, :, 0:2, 2:4], axis=Ax.XY)
                nc.vector.reduce_sum(t2[:], e4[:, :, 2:4, 0:2], axis=Ax.XY)
                nc.vector.tensor_sub(c1[:], t1[:], t2[:])
                t3 = tmp.tile([P, nj], f32, tag=f"t3{tag}")
                t4 = tmp.tile([P, nj], f32, tag=f"t4{tag}")
                nc.vector.reduce_sum(t3[:], e4[:, :, 1, :], axis=Ax.X)
                nc.vector.reduce_sum(t4[:], e4[:, :, 2, :], axis=Ax.X)
                nc.vector.tensor_sub(c2[:], t3[:], t4[:])
                f = tmp.tile([P, nj, 7], f32, tag=f"f{tag}")
                nc.vector.tensor_sub(f[:], e3[:, :, 1:8], e3[:, :, 14:7:-1])
                u1 = tmp.tile([P, nj], f32, tag=f"u1{tag}")
                u2 = tmp.tile([P, nj], f32, tag=f"u2{tag}")
                nc.vector.tensor_sub(u1[:], f[:, :, 0], f[:, :, 1])
                nc.vector.tensor_add(u2[:], f[:, :, 3], f[:, :, 6])
                nc.vector.tensor_sub(u1[:], u1[:], u2[:])
                nc.vector.scalar_tensor_tensor(
                    c3[:], f[:, :, 5], -2.0, u1[:], op0=Alu.mult, op1=Alu.add
                )
                for ck in (c0, c1, c2, c3):
                    nc.vector.tensor_mul(ck[:], ck[:], r[:])
                return c0, c1, c2, c3

            cs1 = coeffs(w1f, 32, "1")
            cs2 = coeffs(w2p, NJ12, "2")

            def combine(j, a, b, cs, hcol, bsb=None, add_eng="gpsimd",
                        w_eng="vector"):
                """hcol = c0 + c1*a + c2*b + c3*a*b.
                bsb: SBUF copy of b when b is PSUM (else b used directly)."""
                c0, c1, c2, c3 = cs
                brd = bsb if bsb is not None else b
                v = tmp.tile([P, B], bf16, tag=f"v{j % 3}")
                nc.scalar.activation(v[:], b, Act.Identity,
                                     bias=c0[:, j:j + 1], scale=c2[:, j:j + 1])
                t = tmp.tile([P, B], bf16, tag=f"t{j % 3}")
                nc.vector.scalar_tensor_tensor(
                    t[:], a, c3[:, j:j + 1], brd, op0=Alu.mult, op1=Alu.mult)
                w = tmp.tile([P, B], bf16, tag=f"w{j % 3}")
                weng = nc.gpsimd if w_eng == "gpsimd" else nc.vector
                weng.scalar_tensor_tensor(
                    w[:], a, c1[:, j:j + 1], v[:], op0=Alu.mult, op1=Alu.add)
                if hcol is None:
                    return t, w
                if add_eng == "gpsimd":
                    nc.gpsimd.tensor_add(hcol, t[:], w[:])
                else:
                    nc.vector.tensor_add(hcol, t[:], w[:])

            # ---- fused L1->L2 ----
            # h1 eval-cols: col jj = 2*j+s holds h1 values for (s==0 ? a : b)
            # side of L2 j-col j.
            h1 = big.tile([P, 32, B], bf16, tag="h1")
            h2 = big.tile([P, NJ12, B], f8, tag="h2")
            for j in range(NJ12):
                for s in (0, 1):
                    jj = 2 * j + s
                    psA = psum.tile([P, B], f32, tag=f"pA{jj % 2}")
                    psB = psum.tile([P, B], f32, tag=f"pB{jj % 2}")
                    base = 4 * jj * P
                    ohA = ohs[:, base:base + 2 * P].rearrange(
                        "p (k m) -> p k m", k=2)
                    ohB = ohs[:, base + 2 * P:base + 4 * P].rearrange(
                        "p (k m) -> p k m", k=2)
                    nc.tensor.matmul(psA[:], ohA, x8i[:],
                                     start=True, stop=True,
                                     perf_mode=mybir.MatmulPerfMode.DoubleRow)
                    nc.tensor.matmul(psB[:], ohB, x8i[:],
                                     start=True, stop=True,
                                     perf_mode=mybir.MatmulPerfMode.DoubleRow)
                    bsb = tmp.tile([P, B], bf16, tag=f"bsb{jj % 2}")
                    nc.scalar.copy(bsb[:], psB[:])
                    combine(jj, psA[:], psB[:], cs1, h1[:, jj], bsb=bsb)
                combine(j, h1[:, 2 * j], h1[:, 2 * j + 1], cs2, h2[:, j])
                if j % JCH == JCH - 1:
                    k = j // JCH
                    nc.sync.dma_start(
                        h2ch[k][:],
                        h2[:, k * JCH:(k + 1) * JCH].rearrange(
                            "p j b -> p (j b)"))
                    nc.gpsimd.collective_compute(
                        "AllGather", Alu.bypass, replica_groups=g8,
                        ins=[h2ch[k][:]],
                        outs=[g2[k * NCORE * P:(k + 1) * NCORE * P, :]],
                    )

            # ---- L3: bucketed gathers from g2 ----
            cs3 = coeffs(w3p, NJ3, "3")
            iab = big.tile([P, 2 * NS3 // 16], i16, tag="i3")
            nc.sync.dma_start(iab[:], i3_d[:])
            ones = coef.tile([P, 1], bf16, tag="ones")
            nc.vector.memset(ones[:], 1.0)
            gps = []
            for g in range(3):
                gt = psum.tile([1, B], f32, tag=f"gps{g}", name=f"gps{g}")
                gps.append(gt)
            gcnt = [0, 0, 0]
            # ordering guard: pre-touch each gather output buffer with a copy
            # that reads h2, so the scheduler cannot hoist the gathers ahead
            # of the last L12 combines on the GpSimd queue (head-of-line
            # blocking on the AllGather would stall the L12 pipeline).
            for ci, (lvl, js) in enumerate(L3_CALLS):
                abg = abp.tile([P, 2 * len(js), B], f8, tag=f"ab{ci}")
                nc.scalar.copy(abg[0:1, 0, 0:1], h2[0:1, 15, 0:1])
            col = 0
            for ci, (lvl, js) in enumerate(L3_CALLS):
                jpc = len(js)
                nidx = 2 * jpc * P
                ncols = nidx // 16
                rows = g2[0:(lvl + 1) * NCORE * P, :].rearrange(
                    "r (q b) -> (r q) b", b=B)
                ab = abp.tile([P, 2 * jpc, B], f8, tag=f"ab{ci}")
                nc.gpsimd.dma_gather(
                    ab[:], rows, iab[:, col:col + ncols],
                    nidx, nidx, B, single_packet=False, queue_num=ci % 4,
                )
                col += ncols
                for jj, j in enumerate(js):
                    t3, w3t = combine(j, ab[:, jj], ab[:, jpc + jj], cs3, None)
                    g = j // JPG3
                    for contrib in (t3, w3t):
                        nc.tensor.matmul(
                            gps[g][:], ones[:], contrib[:],
                            start=(gcnt[g] == 0),
                            stop=(gcnt[g] == 2 * JPG3 - 1),
                            skip_group_check=True)
                        gcnt[g] += 1

            # ---- group-sum results out of PSUM ----
            psc = coef.tile([1, 3 * B], f32, tag="psc")
            for g in range(3):
                nc.scalar.copy(psc[:, g * B:(g + 1) * B], gps[g][:])
            # scale by 1/TAU and fold in the (all-zero) warm-up output
            # before the AllReduce, which then writes out_d directly
            pss = coef.tile([1, 3 * B], f32, tag="pss")
            nc.scalar.mul(pss[:], psc[:], 1.0 / TAU / NCORE * NCORE)
            wsb2 = coef.tile([1, 16], f32, tag="wsb2")
            nc.sync.dma_start(wsb2[:], warm[0:1, :])
            nc.vector.tensor_add(pss[:, :16], pss[:, :16], wsb2[:])
            nc.sync.dma_start(pin[:], pss[:])
            nc.gpsimd.collective_compute(
                "AllReduce", Alu.add, replica_groups=g8,
                ins=[pin[:]], outs=[pout[:]],
            )
            nc.sync.dma_start(out_d[:], pout[:])

    nc.compile()
    return nc


# ---------------- host-side packing (integer/layout only) ----------------

PAD_ROW = np.full(16, -20.0, dtype=np.float32)
PAD_ROW[0] = 20.0


def _wrap_idx(ii):
    w = ii.astype(np.int16).reshape(-1, 16).T
    return np.ascontiguousarray(np.tile(w, (8, 1)))


def _pack_w(w_eff, nj):
    return np.ascontiguousarray(
        w_eff.reshape(nj, P, 16).transpose(1, 0, 2).reshape(P, nj * 16))


def _l3_counts():
    return np.array([667] * 5 + [666] * 3)


def _bucketize(bmax, caps):
    nb = len(caps)
    fill = [0] * nb
    out = np.empty(len(bmax), dtype=np.int64)
    order = np.argsort(bmax, kind="stable")
    for gi in order:
        b = int(bmax[gi])
        while b < nb and fill[b] >= caps[b]:
            b += 1
        assert b < nb, "bucket overflow"
        out[gi] = b
        fill[b] += 1
    return out, fill


def _fit_caps(suffix_need, njcols):
    nb = len(suffix_need)
    caps = [0] * nb
    alloc = 0
    for s in range(nb - 1, 0, -1):
        need = int(np.ceil(suffix_need[s] / P))
        caps[s] = max(0, need - alloc)
        alloc += caps[s]
    caps[0] = njcols - alloc
    if caps[0] < 0:
        return None
    return caps


def _compute_layout(inputs):
    i3a = np.asarray(inputs["idx3a"]).astype(np.int64)
    i3b = np.asarray(inputs["idx3b"]).astype(np.int64)

    # L2 slots natural: gate o -> core o//GPC, slot o%GPC; chunk = j//JCH
    def chunk_l2(i):
        return ((i % GPC) // P) // JCH

    cnts3 = _l3_counts()
    offs3 = np.concatenate([[0], np.cumsum(cnts3)[:-1]])
    bmax3 = np.maximum(chunk_l2(i3a), chunk_l2(i3b))
    need3 = np.zeros(NCH, dtype=np.int64)
    for c in range(NCORE):
        for g in range(3):
            gsel = g * SPG + offs3[c] + np.arange(cnts3[c])
            bm = bmax3[gsel]
            for s in range(NCH):
                need3[s] = max(need3[s], int((bm >= s).sum()))
    caps3 = _fit_caps(need3, JPG3)
    assert caps3 is not None, f"L3 bucket caps infeasible: {need3}"
    return dict(caps3=tuple(caps3), bmax3=bmax3, cnts3=cnts3, offs3=offs3)


def _host_pack(inputs, lay):
    x = np.asarray(inputs["x"], dtype=np.float32)
    w1 = np.asarray(inputs["w1"], dtype=np.float32)
    w2 = np.asarray(inputs["w2"], dtype=np.float32)
    w3 = np.asarray(inputs["w3"], dtype=np.float32)
    i1a = np.asarray(inputs["idx1a"]).astype(np.int64)
    i1b = np.asarray(inputs["idx1b"]).astype(np.int64)
    i2a = np.asarray(inputs["idx2a"]).astype(np.int64)
    i2b = np.asarray(inputs["idx2b"]).astype(np.int64)
    i3a = np.asarray(inputs["idx3a"]).astype(np.int64)
    i3b = np.asarray(inputs["idx3b"]).astype(np.int64)

    import ml_dtypes
    xT = np.zeros((XROWS, B), dtype=np.float32)
    xT[:IN] = x.T

    caps3 = lay["caps3"]
    l3_calls = _l3_call_plan(caps3)

    def row_l2(i):
        c = i // GPC
        t = i - c * GPC
        j = t // P
        p = t - j * P
        k = j // JCH
        return ((k * NCORE + c) * P + p) * JCH + (j % JCH)

    cnts3, offs3 = lay["cnts3"], lay["offs3"]

    in_maps = []
    for c in range(NCORE):
        m = {"xT": xT}
        sel = np.arange(c * GPC, (c + 1) * GPC)

        # ---- fused L1 eval-cols: jj = 2*j+s; slot (jj,p) evaluates L1 gate
        # g1 = idx2{a,b}[core gate j*P+p]; one-hot over x rows ----
        # eval col jj, stream A (i1a) at tile (jj*2+0), stream B at (jj*2+1),
        # each with 2 k-chunks -> col block ((jj*2+st)*2+kc)*P
        w1f_eff = np.tile(PAD_ROW, (32 * P, 1))
        oh = np.zeros((P, 128 * P), dtype=np.float32)
        for j in range(NJ12):
            for s, l2idx in ((0, i2a), (1, i2b)):
                jj = 2 * j + s
                for mm in range(P):
                    lg = j * P + mm          # local L2 gate
                    if lg >= GPC:
                        continue
                    g1 = int(l2idx[c * GPC + lg])   # L1 gate feeding this side
                    w1f_eff[jj * P + mm] = w1[g1]
                    for st, l1idx in ((0, i1a), (1, i1b)):
                        kc, kk = divmod(int(l1idx[g1]), P)
                        oh[kk, ((jj * 2 + st) * 2 + kc) * P + mm] = 1.0
        m["w1f"] = _pack_w(w1f_eff, 32)
        m["oh"] = oh.astype(ml_dtypes.float8_e4m3fn)

        # ---- L2 coeffs (natural slots) ----
        w2_eff = np.concatenate(
            [w2[sel], np.tile(PAD_ROW, (NS12 - GPC, 1))], axis=0)
        m["w2p"] = _pack_w(w2_eff, NJ12)

        # ---- L3: group-aligned, bucketed within group ----
        n_c = cnts3[c]
        w3_eff = np.tile(PAD_ROW, (NS3, 1))
        ia3 = np.zeros(NS3, dtype=np.int64)
        ib3 = np.zeros(NS3, dtype=np.int64)
        l3caps_slots = [cc * P for cc in caps3]
        base_b = np.concatenate([[0], np.cumsum(l3caps_slots)[:-1]])
        for g in range(3):
            gsel = g * SPG + offs3[c] + np.arange(n_c)
            buck, _ = _bucketize(lay["bmax3"][gsel], l3caps_slots)
            cnt = [0] * NCH
            for gi in range(n_c):
                b = buck[gi]
                slot = g * JPG3 * P + base_b[b] + cnt[b]
                cnt[b] += 1
                o = gsel[gi]
                w3_eff[slot] = w3[o]
                ia3[slot] = row_l2(i3a[o])
                ib3[slot] = row_l2(i3b[o])
        m["w3p"] = _pack_w(w3_eff, NJ3)
        parts = []
        for lvl, js in l3_calls:
            jsa = np.concatenate([np.arange(j * P, (j + 1) * P) for j in js])
            parts.append(ia3[jsa])
            parts.append(ib3[jsa])
        m["i3"] = _wrap_idx(np.concatenate(parts))

        in_maps.append(m)
    return in_maps


LAST_RESULTS = None


def kernel(**inputs):
    global LAST_RESULTS
    from concourse.bass_utils import run_bass_kernel_spmd

    lay = _compute_layout(inputs)
    key = lay["caps3"]
    if _CACHE.get("key") != key:
        _CACHE["nc"] = _build_nc(lay["caps3"])
        _CACHE["key"] = key
    nc = _CACHE["nc"]

    in_maps = _host_pack(inputs, lay)
    trace = bool(int(os.environ.get("KERNEL_TRACE", "0")))
    res = run_bass_kernel_spmd(
        nc, in_maps, core_ids=list(range(NCORE)), trace=trace)
    LAST_RESULTS = res

    return np.ascontiguousarray(
        res.results[0]["out"].reshape(3, B).T.astype(np.float32))



# revision 21
# speedup vs baseline: 1.2655x; 1.2655x over previous
"""Trainium2 Bass kernel for the 3-layer difflogic net (nn_Net_48610439856713).

Full batch 512 on the free dim, 8-way neuron sharding (2000 gates/core/
layer), fp8 e4m3 activations.

Per-gate-column combine h = c0 + c1*a + c2*b + c3*ab is split as
  s = c3*a + c2   -- Scalar Act (doubles as the PSUM->SBUF move: the ISA
                     allows at most ONE PSUM stream per DVE op and none
                     on GpSimd, which is TensorTensor-only)
  w = c1*a + c0   -- Vector tensor_scalar (two per-partition scalar APs)
  u = s * b       -- Vector TT, PAIR-BATCHED [128, 2*512] (both sides of
                     an L2 gate in one op; b stays in PSUM)
  h = u + w       -- GpSimd TT add, pair-batched
L2 (all-SBUF): u2 = affine_mul_reduce(h1a, h1b) in ONE Vector op, w2 on
Scalar, h2 add on Vector -> fp8.  L2 emission lags TWO pairs so the
Vector queue never waits on the GpSimd finisher.

Structure:
- Layers 1+2 FUSED: x is replicated, so each core evaluates the L1 values
  its own L2 gates consume, in consumption order, via one-hot PE matmuls
  (fp8 stationaries x fp8 x -> PSUM f32).  L2 emission lags one column
  pair so the Vector queue never stalls.
- L2->L3: 2-chunk 8-rank AllGather (Shared output) overlapped with the
  producer; L3 gates are host-bucketed by max source chunk so gathers
  start after AG chunk 0 lands; gathers run as 4 dma_gather calls on 4
  SWDGE queues.
- L3: u3 = affine_mul_reduce (Vector, SBUF fp8 gathers) and w3 (Scalar)
  pack into fp8 pairs; ONE ones-DoubleRow matmul per column accumulates
  the group sum directly in PSUM.  1/TAU is applied in the PSUM->SBUF
  copies; final tiny AllReduce.

Host-side work is integer/layout bookkeeping only (slot layouts, index
relabeling, one-hot packing, int16 wrapping); all float arithmetic
(softmax coefficients, combines, sums) runs on device.
"""

import os
import numpy as np

P = 128
B = 512
NCORE = 8
IN = 193
XROWS = 256
TAU = 100.0

GPC = 2000
NJ12 = 16
NS12 = NJ12 * P
L3N = 15999
SPG = 5333
NJ3 = 18
JPG3 = 6
NS3 = NJ3 * P

NCH = 2                     # AG chunks for the L2->L3 boundary (8 j-cols)
JCH = NJ12 // NCH

_CACHE = {}


def _l3_call_plan(caps3):
    """caps3: per-group j-cols per level (sum JPG3). Returns list of
    (level, j_list) gather calls; level-0 split so calls stay <=4."""
    base = np.concatenate([[0], np.cumsum(caps3)[:-1]])
    calls = []
    for lvl in range(NCH):
        js = []
        for g in range(3):
            js.extend(range(g * JPG3 + base[lvl], g * JPG3 + base[lvl] + caps3[lvl]))
        if not js:
            continue
        if len(js) > 6:
            third = (len(js) + 2) // 3
            for i in range(0, len(js), third):
                calls.append((lvl, js[i:i + third]))
        else:
            calls.append((lvl, js))
    return calls


def _build_nc(caps3):
    L3_CALLS = _l3_call_plan(caps3)
    import concourse.bacc as bacc
    import concourse.tile as tile
    import concourse.mybir as mybir

    f32 = mybir.dt.float32
    bf16 = mybir.dt.bfloat16
    f8 = mybir.dt.float8e4
    i16 = mybir.dt.int16
    Alu = mybir.AluOpType
    Act = mybir.ActivationFunctionType
    Ax = mybir.AxisListType
    DR = mybir.MatmulPerfMode.DoubleRow

    nc = bacc.Bacc("TRN2", target_bir_lowering=False, debug=False,
                   num_devices=NCORE, num_swdge_queues=4)

    # ---- I/O ----
    xT_d = nc.dram_tensor("xT", [XROWS, B], f32, kind="ExternalInput")
    # one-hot stationaries: 32 eval-cols x 2 streams x 2 kchunks x 128 cols
    oh_d = nc.dram_tensor("oh", [P, 128 * P], f8, kind="ExternalInput")
    w1f = nc.dram_tensor("w1f", [P, 32 * 16], f32, kind="ExternalInput")
    w2p = nc.dram_tensor("w2p", [P, NJ12 * 16], f32, kind="ExternalInput")
    w3p = nc.dram_tensor("w3p", [P, NJ3 * 16], f32, kind="ExternalInput")
    i3_d = nc.dram_tensor("i3", [P, 2 * NS3 // 16], i16, kind="ExternalInput")
    out_d = nc.dram_tensor("out", [1, 3 * B], f32, kind="ExternalOutput")

    h2ch = [nc.dram_tensor(f"h2c{k}", [P, JCH * B], f8, kind="Internal")
            for k in range(NCH)]
    g2 = nc.dram_tensor("g2", [NCH * NCORE * P, JCH * B], f8,
                        kind="Internal", addr_space="Shared")
    win = nc.dram_tensor("win", [1, 16], f32, kind="Internal")
    warm = nc.dram_tensor("warm", [NCORE, 16], f32, kind="Internal")
    pin = nc.dram_tensor("pin", [1, 3 * B], f32, kind="Internal")
    pout = nc.dram_tensor("pout", [1, 3 * B], f32, kind="Internal")

    g8 = [list(range(NCORE))]

    with tile.TileContext(nc) as tc:
        with (
            tc.tile_pool(name="big", bufs=1) as big,
            tc.tile_pool(name="coef", bufs=1) as coef,
            tc.tile_pool(name="tmp", bufs=1) as tmp,
            tc.tile_pool(name="ab", bufs=1) as abp,
            tc.tile_pool(name="psum", bufs=1, space="PSUM") as psum,
        ):
            # ---- warm-up collective ----
            wsb = coef.tile([1, 16], f32, tag="wsb")
            nc.vector.memset(wsb[:], 0.0)
            nc.sync.dma_start(win[:], wsb[:])
            nc.gpsimd.collective_compute(
                "AllGather", Alu.bypass, replica_groups=g8,
                ins=[win[:]], outs=[warm[:]],
            )

            # ---- loads ----
            xt0 = big.tile([P, B], f32, tag="xt0")
            nc.sync.dma_start(xt0[:], xT_d[0:P, :])
            xt1 = big.tile([P, B], f32, tag="xt1")
            nc.sync.dma_start(xt1[:], xT_d[P:XROWS, :])
            x8i = big.tile([P, 2, B], f8, tag="x8i")
            nc.vector.tensor_copy(x8i[:, 0], xt0[:])
            nc.vector.tensor_copy(x8i[:, 1], xt1[:])
            ohs = big.tile([P, 128 * P], f8, tag="ohs")
            for q in range(8):
                nc.sync.dma_start(ohs[:, q * 16 * P:(q + 1) * 16 * P],
                                  oh_d[:, q * 16 * P:(q + 1) * 16 * P])

            def coeffs(wp, nj, tag):
                wt = tmp.tile([P, nj * 16], f32, tag=f"wt{tag}")
                nc.sync.dma_start(wt[:], wp[:])
                e = tmp.tile([P, nj * 16], f32, tag=f"e{tag}")
                nc.scalar.activation(e[:], wt[:], Act.Exp)
                e3 = e[:].rearrange("p (j g) -> p j g", g=16)
                e4 = e[:].rearrange("p (j h q) -> p j h q", h=4, q=4)
                ssum = coef.tile([P, nj], f32, tag=f"ss{tag}")
                nc.vector.reduce_sum(ssum[:], e3, axis=Ax.X)
                r = coef.tile([P, nj], f32, tag=f"r{tag}")
                nc.vector.reciprocal(r[:], ssum[:])
                c0 = coef.tile([P, nj], f32, tag=f"c0{tag}")
                c1 = coef.tile([P, nj], f32, tag=f"c1{tag}")
                c2 = coef.tile([P, nj], f32, tag=f"c2{tag}")
                c3 = coef.tile([P, nj], f32, tag=f"c3{tag}")
                nc.vector.reduce_sum(c0[:], e4[:, :, 2:4, :], axis=Ax.XY)
                t1 = tmp.tile([P, nj], f32, tag=f"t1{tag}")
                t2 = tmp.tile([P, nj], f32, tag=f"t2{tag}")
                nc.vector.reduce_sum(t1[:], e4[:, :, 0:2, 2:4], axis=Ax.XY)
                nc.vector.reduce_sum(t2[:], e4[:, :, 2:4, 0:2], axis=Ax.XY)
                nc.vector.tensor_sub(c1[:], t1[:], t2[:])
                t3 = tmp.tile([P, nj], f32, tag=f"t3{tag}")
                t4 = tmp.tile([P, nj], f32, tag=f"t4{tag}")
                nc.vector.reduce_sum(t3[:], e4[:, :, 1, :], axis=Ax.X)
                nc.vector.reduce_sum(t4[:], e4[:, :, 2, :], axis=Ax.X)
                nc.vector.tensor_sub(c2[:], t3[:], t4[:])
                f = tmp.tile([P, nj, 7], f32, tag=f"f{tag}")
                nc.vector.tensor_sub(f[:], e3[:, :, 1:8], e3[:, :, 14:7:-1])
                u1 = tmp.tile([P, nj], f32, tag=f"u1{tag}")
                u2 = tmp.tile([P, nj], f32, tag=f"u2{tag}")
                nc.vector.tensor_sub(u1[:], f[:, :, 0], f[:, :, 1])
                nc.vector.tensor_add(u2[:], f[:, :, 3], f[:, :, 6])
                nc.vector.tensor_sub(u1[:], u1[:], u2[:])
                nc.vector.scalar_tensor_tensor(
                    c3[:], f[:, :, 5], -2.0, u1[:], op0=Alu.mult, op1=Alu.add
                )
                for ck in (c0, c1, c2, c3):
                    nc.vector.tensor_mul(ck[:], ck[:], r[:])
                return c0, c1, c2, c3

            cs1 = coeffs(w1f, 32, "1")
            cs2 = coeffs(w2p, NJ12, "2")

            # amr scratch (accumulator output, unused)
            scr = coef.tile([P, 1], f32, tag="scr")

            c0_1, c1_1, c2_1, c3_1 = cs1
            c0_2, c1_2, c2_2, c3_2 = cs2

            # ---- fused L1->L2, software-pipelined (L2 lags two pairs) ----
            # h1 eval-cols: col jj = 2*j+s holds h1 values for (s==0 ? a : b)
            # side of L2 j-col j; both sides of a pair share [P, 2, B] tiles.
            h2 = big.tile([P, NJ12, B], f8, tag="h2")
            h1p = [big.tile([P, 2, B], bf16, tag=f"h1p{k}", name=f"h1p{k}")
                   for k in range(3)]
            s1p = [big.tile([P, 2, B], bf16, tag=f"s1p{k}", name=f"s1p{k}")
                   for k in range(3)]
            wp = [big.tile([P, 2, B], bf16, tag=f"wp{k}", name=f"wp{k}")
                  for k in range(3)]
            up = [big.tile([P, 2, B], bf16, tag=f"up{k}", name=f"up{k}")
                  for k in range(3)]
            psA = [[psum.tile([P, B], f32, tag=f"pA{s}{k}", name=f"pA{s}{k}")
                    for k in range(2)] for s in range(2)]
            psB = [psum.tile([P, 2, B], f32, tag=f"pB{k}", name=f"pB{k}")
                   for k in range(2)]

            def emit_l2(j):
                hp = h1p[j % 3]
                u2t = tmp.tile([P, B], bf16, tag=f"u2t{j % 2}",
                               name=f"u2t{j % 2}")
                nc.vector.affine_mul_reduce(
                    u2t[:], scr[:], hp[:, 0], hp[:, 1],
                    c3_2[:, j:j + 1], c2_2[:, j:j + 1])
                w2t = tmp.tile([P, B], bf16, tag=f"w2t{j % 2}",
                               name=f"w2t{j % 2}")
                nc.scalar.activation(
                    w2t[:], hp[:, 0], Act.Identity,
                    bias=c0_2[:, j:j + 1], scale=c1_2[:, j:j + 1])
                nc.vector.tensor_add(h2[:, j], u2t[:], w2t[:])
                if j % JCH == JCH - 1:
                    k = j // JCH
                    nc.sync.dma_start(
                        h2ch[k][:],
                        h2[:, k * JCH:(k + 1) * JCH].rearrange(
                            "p j b -> p (j b)"))
                    nc.gpsimd.collective_compute(
                        "AllGather", Alu.bypass, replica_groups=g8,
                        ins=[h2ch[k][:]],
                        outs=[g2[k * NCORE * P:(k + 1) * NCORE * P, :]],
                    )

            for j in range(NJ12):
                if j > 1:
                    emit_l2(j - 2)
                for s in (0, 1):
                    jj = 2 * j + s
                    base = 4 * jj * P
                    ohA = ohs[:, base:base + 2 * P].rearrange(
                        "p (k m) -> p k m", k=2)
                    ohB = ohs[:, base + 2 * P:base + 4 * P].rearrange(
                        "p (k m) -> p k m", k=2)
                    pa = psA[s][j % 2]
                    nc.tensor.matmul(pa[:], ohA, x8i[:],
                                     start=True, stop=True, perf_mode=DR)
                    nc.tensor.matmul(psB[j % 2][:, s], ohB, x8i[:],
                                     start=True, stop=True, perf_mode=DR)
                    # s = c3*a + c2 (Scalar Act; doubles as PSUM->SBUF move)
                    nc.scalar.activation(
                        s1p[j % 3][:, s], pa[:], Act.Identity,
                        bias=c2_1[:, jj:jj + 1], scale=c3_1[:, jj:jj + 1])
                    # w = c1*a + c0 (Vector, two per-partition scalars)
                    nc.vector.tensor_scalar(
                        wp[j % 3][:, s], pa[:],
                        c1_1[:, jj:jj + 1], c0_1[:, jj:jj + 1],
                        op0=Alu.mult, op1=Alu.add)
                # u = s*b for BOTH sides in one pair-batched TT (b in PSUM)
                nc.vector.tensor_mul(up[j % 3][:], s1p[j % 3][:],
                                     psB[j % 2][:])
                # h1 = u + w, pair-batched on GpSimd
                nc.gpsimd.tensor_add(h1p[j % 3][:], up[j % 3][:],
                                     wp[j % 3][:])
            emit_l2(NJ12 - 2)
            emit_l2(NJ12 - 1)

            # ---- L3: bucketed gathers from g2 ----
            cs3 = coeffs(w3p, NJ3, "3")
            c0_3, c1_3, c2_3, c3_3 = cs3
            iab = big.tile([P, 2 * NS3 // 16], i16, tag="i3")
            nc.sync.dma_start(iab[:], i3_d[:])
            # ones stationary: DoubleRow LDW requires the full 128-wide
            # stationary; every output partition gets the same group sum and
            # partition 0 is read out.  Accumulators reuse the L12 psA banks.
            ones = coef.tile([P, 2, P], f8, tag="ones")
            nc.vector.memset(ones[:], 1.0)
            gps = [psA[0][0][:, :], psA[0][1][:, :], psA[1][0][:, :]]
            gcnt = [0, 0, 0]
            gtot = [0, 0, 0]
            for lvl, js in L3_CALLS:
                for j in js:
                    gtot[j // JPG3] += 1
            # ordering guard: pre-touch each gather output buffer with a copy
            # that reads h2, so the scheduler cannot hoist the gathers ahead
            # of the last L12 work on the in-order GpSimd queue.
            for ci, (lvl, js) in enumerate(L3_CALLS):
                abg = abp.tile([P, 2 * len(js), B], f8, tag=f"ab{ci}",
                               name=f"ab{ci}")
                nc.scalar.copy(abg[0:1, 0, 0:1], h2[0:1, 15, 0:1])
            col = 0
            uw3s = [big.tile([P, 2, B], f8, tag=f"uw3{k}", name=f"uw3{k}")
                    for k in range(3)]
            ei = 0
            for ci, (lvl, js) in enumerate(L3_CALLS):
                jpc = len(js)
                nidx = 2 * jpc * P
                ncols = nidx // 16
                rows = g2[0:(lvl + 1) * NCORE * P, :].rearrange(
                    "r (q b) -> (r q) b", b=B)
                ab = abp.tile([P, 2 * jpc, B], f8, tag=f"ab{ci}",
                              name=f"abx{ci}")
                nc.gpsimd.dma_gather(
                    ab[:], rows, iab[:, col:col + ncols],
                    nidx, nidx, B, single_packet=False, queue_num=ci % 4,
                )
                col += ncols
                for jj, j in enumerate(js):
                    g = j // JPG3
                    uw3 = uw3s[ei % 3]
                    ei += 1
                    nc.vector.affine_mul_reduce(
                        uw3[:, 0], scr[:], ab[:, jj], ab[:, jpc + jj],
                        c3_3[:, j:j + 1], c2_3[:, j:j + 1])
                    nc.scalar.activation(
                        uw3[:, 1], ab[:, jj], Act.Identity,
                        bias=c0_3[:, j:j + 1], scale=c1_3[:, j:j + 1])
                    nc.tensor.matmul(
                        gps[g], ones[:], uw3[:],
                        start=(gcnt[g] == 0),
                        stop=(gcnt[g] == gtot[g] - 1),
                        perf_mode=DR, skip_group_check=True)
                    gcnt[g] += 1

            # ---- group-sum results out of PSUM (scale by 1/TAU here) ----
            psc = coef.tile([1, 3 * B], f32, tag="psc")
            for g in range(3):
                nc.scalar.activation(psc[:, g * B:(g + 1) * B],
                                     gps[g][0:1, :], Act.Identity,
                                     scale=1.0 / TAU)
            # fold in the (all-zero) warm-up output before the AllReduce
            wsb2 = coef.tile([1, 16], f32, tag="wsb2")
            nc.sync.dma_start(wsb2[:], warm[0:1, :])
            nc.vector.tensor_add(psc[:, :16], psc[:, :16], wsb2[:])
            nc.sync.dma_start(pin[:], psc[:])
            nc.gpsimd.collective_compute(
                "AllReduce", Alu.add, replica_groups=g8,
                ins=[pin[:]], outs=[pout[:]],
            )
            nc.sync.dma_start(out_d[:], pout[:])

    nc.compile()
    return nc


# ---------------- host-side packing (integer/layout only) ----------------

PAD_ROW = np.full(16, -20.0, dtype=np.float32)
PAD_ROW[0] = 20.0


def _wrap_idx(ii):
    w = ii.astype(np.int16).reshape(-1, 16).T
    return np.ascontiguousarray(np.tile(w, (8, 1)))


def _pack_w(w_eff, nj):
    return np.ascontiguousarray(
        w_eff.reshape(nj, P, 16).transpose(1, 0, 2).reshape(P, nj * 16))


def _l3_counts():
    return np.array([667] * 5 + [666] * 3)


def _bucketize(bmax, caps):
    nb = len(caps)
    fill = [0] * nb
    out = np.empty(len(bmax), dtype=np.int64)
    order = np.argsort(bmax, kind="stable")
    for gi in order:
        b = int(bmax[gi])
        while b < nb and fill[b] >= caps[b]:
            b += 1
        assert b < nb, "bucket overflow"
        out[gi] = b
        fill[b] += 1
    return out, fill


def _fit_caps(suffix_need, njcols):
    nb = len(suffix_need)
    caps = [0] * nb
    alloc = 0
    for s in range(nb - 1, 0, -1):
        need = int(np.ceil(suffix_need[s] / P))
        caps[s] = max(0, need - alloc)
        alloc += caps[s]
    caps[0] = njcols - alloc
    if caps[0] < 0:
        return None
    return caps


def _compute_layout(inputs):
    i3a = np.asarray(inputs["idx3a"]).astype(np.int64)
    i3b = np.asarray(inputs["idx3b"]).astype(np.int64)

    # L2 slots natural: gate o -> core o//GPC, slot o%GPC; chunk = j//JCH
    def chunk_l2(i):
        return ((i % GPC) // P) // JCH

    cnts3 = _l3_counts()
    offs3 = np.concatenate([[0], np.cumsum(cnts3)[:-1]])
    bmax3 = np.maximum(chunk_l2(i3a), chunk_l2(i3b))
    need3 = np.zeros(NCH, dtype=np.int64)
    for c in range(NCORE):
        for g in range(3):
            gsel = g * SPG + offs3[c] + np.arange(cnts3[c])
            bm = bmax3[gsel]
            for s in range(NCH):
                need3[s] = max(need3[s], int((bm >= s).sum()))
    caps3 = _fit_caps(need3, JPG3)
    assert caps3 is not None, f"L3 bucket caps infeasible: {need3}"
    return dict(caps3=tuple(caps3), bmax3=bmax3, cnts3=cnts3, offs3=offs3)


def _host_pack(inputs, lay):
    x = np.asarray(inputs["x"], dtype=np.float32)
    w1 = np.asarray(inputs["w1"], dtype=np.float32)
    w2 = np.asarray(inputs["w2"], dtype=np.float32)
    w3 = np.asarray(inputs["w3"], dtype=np.float32)
    i1a = np.asarray(inputs["idx1a"]).astype(np.int64)
    i1b = np.asarray(inputs["idx1b"]).astype(np.int64)
    i2a = np.asarray(inputs["idx2a"]).astype(np.int64)
    i2b = np.asarray(inputs["idx2b"]).astype(np.int64)
    i3a = np.asarray(inputs["idx3a"]).astype(np.int64)
    i3b = np.asarray(inputs["idx3b"]).astype(np.int64)

    import ml_dtypes
    xT = np.zeros((XROWS, B), dtype=np.float32)
    xT[:IN] = x.T

    caps3 = lay["caps3"]
    l3_calls = _l3_call_plan(caps3)

    def row_l2(i):
        c = i // GPC
        t = i - c * GPC
        j = t // P
        p = t - j * P
        k = j // JCH
        return ((k * NCORE + c) * P + p) * JCH + (j % JCH)

    cnts3, offs3 = lay["cnts3"], lay["offs3"]

    in_maps = []
    for c in range(NCORE):
        m = {"xT": xT}
        sel = np.arange(c * GPC, (c + 1) * GPC)

        # ---- fused L1 eval-cols: jj = 2*j+s; slot (jj,p) evaluates L1 gate
        # g1 = idx2{a,b}[core gate j*P+p]; one-hot over x rows ----
        # eval col jj, stream A (i1a) at tile (jj*2+0), stream B at (jj*2+1),
        # each with 2 k-chunks -> col block ((jj*2+st)*2+kc)*P
        w1f_eff = np.tile(PAD_ROW, (32 * P, 1))
        oh = np.zeros((P, 128 * P), dtype=np.float32)
        for j in range(NJ12):
            for s, l2idx in ((0, i2a), (1, i2b)):
                jj = 2 * j + s
                for mm in range(P):
                    lg = j * P + mm          # local L2 gate
                    if lg >= GPC:
                        continue
                    g1 = int(l2idx[c * GPC + lg])   # L1 gate feeding this side
                    w1f_eff[jj * P + mm] = w1[g1]
                    for st, l1idx in ((0, i1a), (1, i1b)):
                        kc, kk = divmod(int(l1idx[g1]), P)
                        oh[kk, ((jj * 2 + st) * 2 + kc) * P + mm] = 1.0
        m["w1f"] = _pack_w(w1f_eff, 32)
        m["oh"] = oh.astype(ml_dtypes.float8_e4m3fn)

        # ---- L2 coeffs (natural slots) ----
        w2_eff = np.concatenate(
            [w2[sel], np.tile(PAD_ROW, (NS12 - GPC, 1))], axis=0)
        m["w2p"] = _pack_w(w2_eff, NJ12)

        # ---- L3: group-aligned, bucketed within group ----
        n_c = cnts3[c]
        w3_eff = np.tile(PAD_ROW, (NS3, 1))
        ia3 = np.zeros(NS3, dtype=np.int64)
        ib3 = np.zeros(NS3, dtype=np.int64)
        l3caps_slots = [cc * P for cc in caps3]
        base_b = np.concatenate([[0], np.cumsum(l3caps_slots)[:-1]])
        for g in range(3):
            gsel = g * SPG + offs3[c] + np.arange(n_c)
            buck, _ = _bucketize(lay["bmax3"][gsel], l3caps_slots)
            cnt = [0] * NCH
            for gi in range(n_c):
                b = buck[gi]
                slot = g * JPG3 * P + base_b[b] + cnt[b]
                cnt[b] += 1
                o = gsel[gi]
                w3_eff[slot] = w3[o]
                ia3[slot] = row_l2(i3a[o])
                ib3[slot] = row_l2(i3b[o])
        m["w3p"] = _pack_w(w3_eff, NJ3)
        parts = []
        for lvl, js in l3_calls:
            jsa = np.concatenate([np.arange(j * P, (j + 1) * P) for j in js])
            parts.append(ia3[jsa])
            parts.append(ib3[jsa])
        m["i3"] = _wrap_idx(np.concatenate(parts))

        in_maps.append(m)
    return in_maps


LAST_RESULTS = None


def kernel(**inputs):
    global LAST_RESULTS
    from concourse.bass_utils import run_bass_kernel_spmd

    lay = _compute_layout(inputs)
    key = lay["caps3"]
    if _CACHE.get("key") != key:
        _CACHE["nc"] = _build_nc(lay["caps3"])
        _CACHE["key"] = key
    nc = _CACHE["nc"]

    in_maps = _host_pack(inputs, lay)
    trace = bool(int(os.environ.get("KERNEL_TRACE", "0")))
    res = run_bass_kernel_spmd(
        nc, in_maps, core_ids=list(range(NCORE)), trace=trace)
    LAST_RESULTS = res

    return np.ascontiguousarray(
        res.results[0]["out"].reshape(3, B).T.astype(np.float32))
